# revision 24
# baseline (speedup 1.0000x reference)
"""Trainium2 Bass kernel for nn_EquivariantLayer (gnn_message_passing).

Computes, per batch element:  out = x @ A - ones(N,1) @ (colsum(x) @ B)
with x [65536, 64] f32, A/B [64, 64] f32.

Sharding: batch axis (8) -> 8 NeuronCores, A/B replicated; no collectives.

Per-core roofline: read 16.78 MB of x, write 8.39 MB fp16 out; output
depends on colsum(x) so the two DMA phases serialize -> ~70 us floor at
~358 GB/s.  The PE never leaves its cold 1.2 GHz clock for N=128 matmul
streams (HAM), so the design keeps PE off the critical path entirely:

  Phase 1 (streaming 16 tiles of 4096 rows, DMA-bound ~47 us):
    - SWDGE (gpsimd) DMA casts x f32 -> fp16 inline
    - DVE pairwise-folds each fp16 tile along free axis (2x mode) into
      per-tile partial colsums (f32 last level)
    - PE pair-transposes [128,128] fp16 blocks into PSUM; ACT evicts to
      rolling x^T tiles
    - PE matmuls x^T blocks vs block-diag [[A,0],[0,A]] fp16 -> PSUM;
      ACT evicts x@A as fp16 into persistent park tiles (no s needed!)
    - last `nt - defer_from` tiles' matmuls spill into phase 2 (PE is
      idle there; parks stay ahead of the ordered out-DMA stream)
  Interlude: stats -> s (PE ones-matmul) -> -s@B -> fp16 bc row [128,64]
  Phase 2 (DMA-bound ~24 us):
    - DVE in-place adds bcast(-s@B) to each park group (all-fp16, 2x)
    - HWDGE streams fp16 park tiles out (512 KB per tile)

Output fp16 (|out| < ~150, fp16 RMS rel err ~2.4e-4); host upcasts.
"""

import sys

for _p in ("/opt/trn_rl_repo",):
    if _p not in sys.path:
        sys.path.insert(0, _p)

import numpy as np

import concourse.bass as bass
import concourse.tile as tile
from concourse import bacc, mybir

F32 = mybir.dt.float32
F16 = mybir.dt.float16

N_CORES = 8
N_ROWS = 65536
C = 64
P = 128


def _bcast_row(ap, reps):
    """[p, C] AP -> [p, reps, C] AP with step-0 middle dim."""
    return bass.AP(
        tensor=ap.tensor,
        offset=ap.offset,
        ap=[list(ap.ap[0]), [0, reps], list(ap.ap[1])],
    )


def build(n_rows=N_ROWS, tile_rows=4096, defer_mod=2):
    assert n_rows % tile_rows == 0
    nt = n_rows // tile_rows          # 16 tiles
    free = tile_rows * C // P         # 2048 fp16 elems per partition
    kb = tile_rows // (2 * P)         # 16 transpose pairs per tile
    assert kb % 8 == 0
    gb = kb // 8                      # 2 groups of [128,1024] per tile

    nc = bacc.Bacc(
        "TRN2", target_bir_lowering=False, debug=False, num_devices=N_CORES
    )
    x_d = nc.dram_tensor("x", [n_rows, C], F32, kind="ExternalInput").ap()
    b_d = nc.dram_tensor("B", [C, C], F32, kind="ExternalInput").ap()
    id_d = nc.dram_tensor("ident", [P, P], F16, kind="ExternalInput").ap()
    a2_d = nc.dram_tensor("A2", [P, P], F16, kind="ExternalInput").ap()
    o_d = nc.dram_tensor("out", [n_rows, C], F16, kind="ExternalOutput").ap()

    with tile.TileContext(nc) as tc:
        with (
            tc.tile_pool(name="consts", bufs=1) as consts,
            tc.tile_pool(name="xbf", bufs=7) as xbf,
            tc.tile_pool(name="scr", bufs=2) as scr,
            tc.tile_pool(name="xtp", bufs=24) as xtp,
            tc.tile_pool(name="parkp", bufs=nt) as parkp,
            tc.tile_pool(name="statsp", bufs=2) as statsp,
            tc.tile_pool(name="tpsum", bufs=2, space="PSUM") as tpsum,
            tc.tile_pool(name="opsum", bufs=3, space="PSUM") as opsum,
        ):
            ident = consts.tile([P, P], F16)
            nc.scalar.dma_start(out=ident[:], in_=id_d)
            a2_sb = consts.tile([P, P], F16)
            nc.scalar.dma_start(out=a2_sb[:], in_=a2_d)
            b_sb = consts.tile([64, C], F32)
            nc.scalar.dma_start(out=b_sb[:], in_=b_d)
            ones_p = consts.tile([P, 1], F32)
            nc.vector.memset(ones_p[:], 1.0)
            ones_m = consts.tile([64, P], F32)
            nc.vector.memset(ones_m[:], 1.0)
            ones1 = consts.tile([1, P], F16)
            nc.vector.memset(ones1[:], 1.0)

            acc = statsp.tile([P, 4 * C], F32)
            nc.vector.memset(acc[:], 0.0)

            parks = []
            deferred = []  # (park, xt_tiles) whose matmuls run late
            nbc16 = consts.tile([P, C], F16)
            sbrhs = consts.tile([1, 512], F16)

            def emit_interlude():
                # acc -> s -> -s@B -> fp16 bc row + K=1 ones-mm rhs.
                # Emitted right after the LAST tile's folds so the s-chain
                # beats that tile's transposes into the in-order PE queue.
                with tc.high_priority(offset=100):
                    sums = consts.tile([P, C], F32)
                    nc.vector.tensor_reduce(
                        out=sums[:],
                        in_=acc[:].rearrange("p (j c) -> p c j", c=C),
                        axis=mybir.AxisListType.X,
                        op=mybir.AluOpType.add,
                    )
                    sp = opsum.tile([P, 1024], F32, tag="ob")
                    nc.tensor.matmul(
                        out=sp[0:64, 0:1], lhsT=sums[:], rhs=ones_p[:],
                        start=True, stop=True,
                    )
                    nst_sb = consts.tile([64, 1], F32)
                    nc.scalar.copy(out=nst_sb[:], in_=sp[0:64, 0:1])
                    nbs_sb = consts.tile([64, C], F32)
                    nc.vector.tensor_scalar(
                        out=nbs_sb[:], in0=b_sb[:], scalar1=nst_sb[:],
                        scalar2=-1.0,
                        op0=mybir.AluOpType.mult, op1=mybir.AluOpType.mult,
                    )
                    sp2 = opsum.tile([P, 1024], F32, tag="ob")
                    # bc = ones (x) -(s@B): [128, 64]
                    nc.tensor.matmul(
                        out=sp2[:, 0:C], lhsT=ones_m[:], rhs=nbs_sb[:],
                        start=True, stop=True,
                    )
                    nc.scalar.copy(out=nbc16[:], in_=sp2[:, 0:C])
                    nc.scalar.copy(
                        out=sbrhs[0:1, :].rearrange("p (r c) -> p r c", c=C),
                        in_=_bcast_row(nbc16[0:1, :], 8),
                    )

            # ---- phase 1 ----
            for t in range(nt):
                xview = x_d[t * tile_rows : (t + 1) * tile_rows, :].rearrange(
                    "(p j) c -> p (j c)", p=P
                )
                xb = xbf.tile([P, free], F16)
                nc.gpsimd.dma_start(out=xb[:], in_=xview)
                # fp16 pairwise folds to 256 elems, then f32 accumulate
                sc = scr.tile([P, free // 2], F16)
                half = free // 2
                with tc.high_priority(offset=50):
                    nc.vector.tensor_add(
                        out=sc[:, 0:half],
                        in0=xb[:, 0:half],
                        in1=xb[:, half : 2 * half],
                    )
                    while half > 4 * C:
                        half //= 2
                        nc.vector.tensor_add(
                            out=sc[:, 0:half],
                            in0=sc[:, 0:half],
                            in1=sc[:, half : 2 * half],
                        )
                    nc.vector.tensor_add(
                        out=acc[:], in0=acc[:], in1=sc[:, 0 : 4 * C]
                    )
                if t == nt - 1:
                    emit_interlude()
                park = parkp.tile([P, free], F16, tag="park")
                xts = []
                for g in range(gb):
                    tb = tpsum.tile([P, 1024], F16, tag="tb")
                    for u in range(8):
                        k = 8 * g + u
                        nc.tensor.transpose(
                            out=tb[:, 128 * u : 128 * u + 128],
                            in_=xb[:, 128 * k : 128 * k + 128],
                            identity=ident[:],
                        )
                    xt_sb = xtp.tile([P, 1024], F16, tag="xt")
                    nc.scalar.copy(
                        out=xt_sb[:].bitcast(F32), in_=tb[:].bitcast(F32)
                    )
                    xts.append(xt_sb)
                if defer_mod and (t % defer_mod == 1 or t in (12, 14)):
                    deferred.append((park, xts))
                else:
                    _emit_mm_park(nc, opsum, xts, park, a2_sb, gb)
                parks.append(park)

            nbc_bcast = _bcast_row(nbc16[:], 16)

            # ---- phase 2 ----
            # deferred tiles fold -s@B into PSUM via K=1 ones-matmuls and
            # get a plain park evict (no DVE pass); phase-1-parked tiles
            # get the in-place DVE add instead
            for park, xts in deferred:
                _emit_mm_park(nc, opsum, xts, park, a2_sb, gb)
            for t in range(nt):
                park = parks[t]
                oview = o_d[t * tile_rows : (t + 1) * tile_rows, :].rearrange(
                    "(p j) c -> p (j c)", p=P
                )
                for g in range(gb):
                    seg = 1024 * g
                    sl = park[:, seg : seg + 1024].rearrange(
                        "p (j c) -> p j c", c=C
                    )
                    nc.vector.tensor_add(out=sl, in0=sl, in1=nbc_bcast)
                    if t == 0:
                        # tile 0 streams out per group: the first bytes
                        # leave right after the first in-place add
                        nc.sync.dma_start(
                            out=oview[:, seg : seg + 1024],
                            in_=park[:, seg : seg + 1024],
                        )
                if t != 0:
                    nc.sync.dma_start(out=oview, in_=park[:])

    nc.compile()
    return nc


def _emit_mm_park(nc, opsum, xts, park, a2_sb, gb, ones1=None, sbrhs=None):
    for g in range(gb):
        ob = opsum.tile([P, 1024], F32, tag="ob")
        xt_sb = xts[g]
        for u in range(8):
            nc.tensor.matmul(
                out=ob[:, 128 * u : 128 * u + 128],
                lhsT=xt_sb[:, 128 * u : 128 * u + 128],
                rhs=a2_sb[:],
                start=(u % 4 == 0),
                stop=(u % 4 == 3) and ones1 is None,
            )
        if ones1 is not None:
            # accumulate -(s@B) into both PSUM banks (K=1 fp16)
            nc.tensor.matmul(
                out=ob[:, 0:512], lhsT=ones1[:], rhs=sbrhs[:],
                start=False, stop=True,
            )
            nc.tensor.matmul(
                out=ob[:, 512:1024], lhsT=ones1[:], rhs=sbrhs[:],
                start=False, stop=True,
            )
        seg = 1024 * g
        nc.scalar.copy(out=park[:, seg : seg + 1024], in_=ob[:])


_CACHE = {}


def _get_compiled():
    if "nc" not in _CACHE:
        _CACHE["nc"] = build()
    return _CACHE["nc"]


def _run(nc, x, A, B, **kwargs):
    import ml_dtypes
    from concourse.bass_utils import run_bass_kernel_spmd

    x = np.ascontiguousarray(np.asarray(x, dtype=np.float32))
    A = np.ascontiguousarray(np.asarray(A, dtype=np.float32))
    B = np.ascontiguousarray(np.asarray(B, dtype=np.float32))
    ident = np.eye(P, dtype=np.float16)
    a2 = np.zeros((P, P), dtype=np.float16)
    a2[0:C, 0:C] = A.astype(np.float16)
    a2[C:P, C:P] = A.astype(np.float16)
    n_cores = x.shape[0]
    in_maps = [
        {"x": x[i], "B": B, "ident": ident, "A2": a2} for i in range(n_cores)
    ]
    res = run_bass_kernel_spmd(nc, in_maps, core_ids=list(range(n_cores)), **kwargs)
    out = np.stack([res.results[i]["out"] for i in range(n_cores)], axis=0)
    return out, res


def kernel(x, A, B):
    nc = _get_compiled()
    out, _ = _run(nc, x, A, B)
    return out.astype(np.float32)


# revision 25
# speedup vs baseline: 1.0637x; 1.0637x over previous
"""Trainium2 Bass kernel for nn_EquivariantLayer (gnn_message_passing).

Computes, per batch element:  out = x @ A - ones(N,1) @ (colsum(x) @ B)
with x [65536, 64] f32, A/B [64, 64] f32.

Sharding: batch axis (8) -> 8 NeuronCores, A/B replicated; no collectives.

Per-core roofline: read 16.78 MB of x, write 8.39 MB fp16 out; output
depends on colsum(x) so the two DMA phases serialize -> ~70 us floor at
~358 GB/s.  The PE never leaves its cold 1.2 GHz clock for N=128 matmul
streams (HAM), so the design keeps PE off the critical path entirely:

  Phase 1 (streaming 16 tiles of 4096 rows, DMA-bound ~47 us):
    - SWDGE (gpsimd) DMA casts x f32 -> fp16 inline
    - DVE pairwise-folds each fp16 tile along free axis (2x mode) into
      per-tile partial colsums (f32 last level)
    - PE pair-transposes [128,128] fp16 blocks into PSUM; ACT evicts to
      rolling x^T tiles
    - PE matmuls x^T blocks vs block-diag [[A,0],[0,A]] fp16 -> PSUM;
      ACT evicts x@A as fp16 into persistent park tiles (no s needed!)
    - last `nt - defer_from` tiles' matmuls spill into phase 2 (PE is
      idle there; parks stay ahead of the ordered out-DMA stream)
  Interlude: stats -> s (PE ones-matmul) -> -s@B -> fp16 bc row [128,64]
  Phase 2 (DMA-bound ~24 us):
    - DVE in-place adds bcast(-s@B) to each park group (all-fp16, 2x)
    - HWDGE streams fp16 park tiles out (512 KB per tile)

Output fp16 (|out| < ~150, fp16 RMS rel err ~2.4e-4); host upcasts.
"""

import sys

for _p in ("/opt/trn_rl_repo",):
    if _p not in sys.path:
        sys.path.insert(0, _p)

import numpy as np

import concourse.bass as bass
import concourse.tile as tile
from concourse import bacc, mybir

F32 = mybir.dt.float32
F16 = mybir.dt.float16

N_CORES = 8
N_ROWS = 65536
C = 64
P = 128


def _bcast_row(ap, reps):
    """[p, C] AP -> [p, reps, C] AP with step-0 middle dim."""
    return bass.AP(
        tensor=ap.tensor,
        offset=ap.offset,
        ap=[list(ap.ap[0]), [0, reps], list(ap.ap[1])],
    )


def build(n_rows=N_ROWS, tile_rows=4096, defer_mod=2):
    assert n_rows % tile_rows == 0
    nt = n_rows // tile_rows          # 16 tiles
    free = tile_rows * C // P         # 2048 fp16 elems per partition
    kb = tile_rows // (2 * P)         # 16 transpose pairs per tile
    assert kb % 8 == 0
    gb = kb // 8                      # 2 groups of [128,1024] per tile

    nc = bacc.Bacc(
        "TRN2", target_bir_lowering=False, debug=False, num_devices=N_CORES
    )
    x_d = nc.dram_tensor("x", [n_rows, C], F32, kind="ExternalInput").ap()
    b_d = nc.dram_tensor("B", [C, C], F32, kind="ExternalInput").ap()
    id_d = nc.dram_tensor("ident", [P, P], F16, kind="ExternalInput").ap()
    a2_d = nc.dram_tensor("A2", [P, P], F16, kind="ExternalInput").ap()
    o_d = nc.dram_tensor("out", [n_rows, C], F16, kind="ExternalOutput").ap()

    with tile.TileContext(nc) as tc:
        with (
            tc.tile_pool(name="consts", bufs=1) as consts,
            tc.tile_pool(name="xbf", bufs=6) as xbf,
            tc.tile_pool(name="scr", bufs=2) as scr,
            tc.tile_pool(name="xtp", bufs=20) as xtp,
            tc.tile_pool(name="parkp", bufs=nt) as parkp,
            tc.tile_pool(name="statsp", bufs=2) as statsp,
            tc.tile_pool(name="tpsum", bufs=2, space="PSUM") as tpsum,
            tc.tile_pool(name="opsum", bufs=3, space="PSUM") as opsum,
        ):
            ident = consts.tile([P, P], F16)
            nc.scalar.dma_start(out=ident[:], in_=id_d)
            a2_sb = consts.tile([P, P], F16)
            nc.scalar.dma_start(out=a2_sb[:], in_=a2_d)
            b_sb = consts.tile([64, C], F32)
            nc.scalar.dma_start(out=b_sb[:], in_=b_d)
            ones_p = consts.tile([P, 1], F32)
            nc.vector.memset(ones_p[:], 1.0)
            ones_m = consts.tile([64, P], F32)
            nc.vector.memset(ones_m[:], 1.0)
            ones1 = consts.tile([1, P], F16)
            nc.vector.memset(ones1[:], 1.0)

            acc = statsp.tile([P, 4 * C], F32)
            nc.vector.memset(acc[:], 0.0)

            parks = []
            deferred = []  # (park, xt_tiles) whose matmuls run late
            nbc16 = consts.tile([P, C], F16)
            sbrhs = consts.tile([1, 512], F16)

            def emit_interlude():
                # acc -> s -> -s@B -> fp16 bc row + K=1 ones-mm rhs.
                # Emitted right after the LAST tile's folds so the s-chain
                # beats that tile's transposes into the in-order PE queue.
                with tc.high_priority(offset=100):
                    sums = consts.tile([P, C], F32)
                    nc.vector.tensor_reduce(
                        out=sums[:],
                        in_=acc[:].rearrange("p (j c) -> p c j", c=C),
                        axis=mybir.AxisListType.X,
                        op=mybir.AluOpType.add,
                    )
                    sp = opsum.tile([P, 1024], F32, tag="ob")
                    nc.tensor.matmul(
                        out=sp[0:64, 0:1], lhsT=sums[:], rhs=ones_p[:],
                        start=True, stop=True,
                    )
                    nst_sb = consts.tile([64, 1], F32)
                    nc.scalar.copy(out=nst_sb[:], in_=sp[0:64, 0:1])
                    nbs_sb = consts.tile([64, C], F32)
                    nc.vector.tensor_scalar(
                        out=nbs_sb[:], in0=b_sb[:], scalar1=nst_sb[:],
                        scalar2=-1.0,
                        op0=mybir.AluOpType.mult, op1=mybir.AluOpType.mult,
                    )
                    sp2 = opsum.tile([P, 1024], F32, tag="ob")
                    # bc = ones (x) -(s@B): [128, 64]
                    nc.tensor.matmul(
                        out=sp2[:, 0:C], lhsT=ones_m[:], rhs=nbs_sb[:],
                        start=True, stop=True,
                    )
                    nc.scalar.copy(out=nbc16[:], in_=sp2[:, 0:C])
                    nc.scalar.copy(
                        out=sbrhs[0:1, :].rearrange("p (r c) -> p r c", c=C),
                        in_=_bcast_row(nbc16[0:1, :], 8),
                    )

            # ---- phase 1 ----
            for t in range(nt):
                xview = x_d[t * tile_rows : (t + 1) * tile_rows, :].rearrange(
                    "(p j) c -> p (j c)", p=P
                )
                xb = xbf.tile([P, free], F16)
                nc.gpsimd.dma_start(out=xb[:], in_=xview)
                # fp16 pairwise folds to 256 elems, then f32 accumulate
                sc = scr.tile([P, free // 2], F16)
                half = free // 2
                with tc.high_priority(offset=50):
                    nc.vector.tensor_add(
                        out=sc[:, 0:half],
                        in0=xb[:, 0:half],
                        in1=xb[:, half : 2 * half],
                    )
                    while half > 4 * C:
                        half //= 2
                        nc.vector.tensor_add(
                            out=sc[:, 0:half],
                            in0=sc[:, 0:half],
                            in1=sc[:, half : 2 * half],
                        )
                    nc.vector.tensor_add(
                        out=acc[:], in0=acc[:], in1=sc[:, 0 : 4 * C]
                    )
                if t == nt - 1:
                    emit_interlude()
                park = parkp.tile([P, free], F16, tag="park")
                xts = []
                for g in range(gb):
                    tb = tpsum.tile([P, 1024], F16, tag="tb")
                    for u in range(8):
                        k = 8 * g + u
                        nc.tensor.transpose(
                            out=tb[:, 128 * u : 128 * u + 128],
                            in_=xb[:, 128 * k : 128 * k + 128],
                            identity=ident[:],
                        )
                    xt_sb = xtp.tile([P, 1024], F16, tag="xt")
                    nc.scalar.copy(
                        out=xt_sb[:].bitcast(F32), in_=tb[:].bitcast(F32)
                    )
                    xts.append(xt_sb)
                if defer_mod and t % defer_mod == 1:
                    deferred.append((park, xts))
                else:
                    _emit_mm_park(nc, opsum, xts, park, a2_sb, gb)
                parks.append(park)

            nbc_bcast = _bcast_row(nbc16[:], 16)

            # ---- phase 2 ----
            # deferred tiles fold -s@B into PSUM via K=1 ones-matmuls and
            # get a plain park evict (no DVE pass); phase-1-parked tiles
            # get the in-place DVE add instead
            for park, xts in deferred:
                _emit_mm_park(nc, opsum, xts, park, a2_sb, gb)
            for t in range(nt):
                park = parks[t]
                oview = o_d[t * tile_rows : (t + 1) * tile_rows, :].rearrange(
                    "(p j) c -> p (j c)", p=P
                )
                for g in range(gb):
                    seg = 1024 * g
                    sl = park[:, seg : seg + 1024].rearrange(
                        "p (j c) -> p j c", c=C
                    )
                    nc.vector.tensor_add(out=sl, in0=sl, in1=nbc_bcast)
                    if t == 0:
                        # tile 0 streams out per group: the first bytes
                        # leave right after the first in-place add
                        nc.sync.dma_start(
                            out=oview[:, seg : seg + 1024],
                            in_=park[:, seg : seg + 1024],
                        )
                if t != 0:
                    nc.sync.dma_start(out=oview, in_=park[:])

    nc.compile()
    return nc


def _emit_mm_park(nc, opsum, xts, park, a2_sb, gb, ones1=None, sbrhs=None):
    for g in range(gb):
        ob = opsum.tile([P, 1024], F32, tag="ob")
        xt_sb = xts[g]
        for u in range(8):
            nc.tensor.matmul(
                out=ob[:, 128 * u : 128 * u + 128],
                lhsT=xt_sb[:, 128 * u : 128 * u + 128],
                rhs=a2_sb[:],
                start=(u % 4 == 0),
                stop=(u % 4 == 3) and ones1 is None,
            )
        if ones1 is not None:
            # accumulate -(s@B) into both PSUM banks (K=1 fp16)
            nc.tensor.matmul(
                out=ob[:, 0:512], lhsT=ones1[:], rhs=sbrhs[:],
                start=False, stop=True,
            )
            nc.tensor.matmul(
                out=ob[:, 512:1024], lhsT=ones1[:], rhs=sbrhs[:],
                start=False, stop=True,
            )
        seg = 1024 * g
        nc.scalar.copy(out=park[:, seg : seg + 1024], in_=ob[:])


_CACHE = {}


def _get_compiled():
    if "nc" not in _CACHE:
        _CACHE["nc"] = build()
    return _CACHE["nc"]


def _run(nc, x, A, B, **kwargs):
    import ml_dtypes
    from concourse.bass_utils import run_bass_kernel_spmd

    x = np.ascontiguousarray(np.asarray(x, dtype=np.float32))
    A = np.ascontiguousarray(np.asarray(A, dtype=np.float32))
    B = np.ascontiguousarray(np.asarray(B, dtype=np.float32))
    ident = np.eye(P, dtype=np.float16)
    a2 = np.zeros((P, P), dtype=np.float16)
    a2[0:C, 0:C] = A.astype(np.float16)
    a2[C:P, C:P] = A.astype(np.float16)
    n_cores = x.shape[0]
    in_maps = [
        {"x": x[i], "B": B, "ident": ident, "A2": a2} for i in range(n_cores)
    ]
    res = run_bass_kernel_spmd(nc, in_maps, core_ids=list(range(n_cores)), **kwargs)
    out = np.stack([res.results[i]["out"] for i in range(n_cores)], axis=0)
    return out, res


def kernel(x, A, B):
    nc = _get_compiled()
    out, _ = _run(nc, x, A, B)
    return out.astype(np.float32)
